# revision 3
# baseline (speedup 1.0000x reference)
"""Trainium2 Bass kernel for nn_CropConvLSTM.

Model: ConvLSTM (Conv1d(1+H -> 4H, k=3, pad=1), S=12 steps) over x (B=256,
S=12, L=128), then head Linear(98304->768)+BN+ReLU, Linear(768->12)+BN+ReLU,
Linear(12->10).

Distribution over 8 NeuronCores, three launches:
  Stage 1: ConvLSTM, data-parallel over batch (32 samples/core). Conv done as
    3 shifted fp32r matmuls (K=66: 64 h-channels + x-row + ones-row for the
    conv bias) accumulating in PSUM; gates on ACT/DVE/GPSIMD with all tensors
    at legal partition bases (tanh computed as 2*sigmoid(2x)-1 so a single
    per-partition-scaled sigmoid covers the [o;g] psum tile).
  Stage 2: y1 = flat @ (w1*bn1_scale).T, sharded over the 98304 contract dim
    (12288 features/core); each core emits a partial (768, 256), host reduces.
  Stage 3: bias+ReLU, Linear2+BN+ReLU, Linear3 (+b3 via ones-row trick),
    data-parallel over batch again.

BN (eval mode) is folded into the weights/biases on the host.
"""
import sys

sys.path.insert(0, "/opt/trn_rl_repo")

from functools import lru_cache

import numpy as np

import concourse.bass as bass
import concourse.tile as tile
from concourse import bacc, mybir
from concourse.bass_utils import run_bass_kernel_spmd

F32 = mybir.dt.float32
F32R = mybir.dt.float32r
AF = mybir.ActivationFunctionType

B, S, L, H, C = 256, 12, 128, 64, 10
NC = 8
BLOC = B // NC            # 32 samples per core in stages 1/3
KTOT = S * H * L          # 98304
KSH = KTOT // NC          # 12288 contract features per core in stage 2
KCH = KSH // 128          # 96 k-chunks per core
EPS = 1e-5
CORE_IDS = list(range(NC))


# ---------------------------------------------------------------- stage 1
@lru_cache(maxsize=1)
def _build_stage1():
    nc = bacc.Bacc("TRN2", target_bir_lowering=False, debug=False, num_devices=NC)
    xp = nc.dram_tensor("xp", [S, BLOC, L], F32R, kind="ExternalInput").ap()
    wfi = nc.dram_tensor("wfi", [66, 3, 128], F32R, kind="ExternalInput").ap()
    wif = nc.dram_tensor("wif", [66, 3, 128], F32R, kind="ExternalInput").ap()
    wog = nc.dram_tensor("wog", [66, 3, 128], F32R, kind="ExternalInput").ap()
    hs = nc.dram_tensor("hs", [S, H, BLOC, L], F32, kind="ExternalOutput").ap()

    HB = BLOC // 2        # 16 samples per half-step
    FH = HB * L           # 2048 free per half

    with tile.TileContext(nc) as tc:
        with (
            tc.tile_pool(name="persist", bufs=1) as pp,
            tc.tile_pool(name="sig", bufs=2) as sp,
            tc.tile_pool(name="tmp", bufs=2) as tp,
            tc.tile_pool(name="psfi", bufs=1, space="PSUM") as ps_fi,
            tc.tile_pool(name="psog", bufs=1, space="PSUM") as ps_og,
        ):
            comb = pp.tile([66, BLOC, L + 2], F32R)     # [h(64); x; ones]
            cpk = pp.tile([128, HB, L], F32)            # c: [h1 | h2] packed
            wt_fi = pp.tile([66, 3, 128], F32R)
            wt_if = pp.tile([66, 3, 128], F32R)
            wt_og = pp.tile([66, 3, 128], F32R)
            scv = pp.tile([128, 1], F32)                # act scale [1;2]

            nc.vector.memset(comb.bitcast(F32), 0.0)
            # ones row is partition 65; engine bases must be 32-aligned, so
            # set [64:66] to 1.0 then zero the x row (64) again
            nc.vector.memset(comb[64:66].bitcast(F32), 1.0)
            nc.vector.memset(comb[64:65].bitcast(F32), 0.0)
            nc.vector.memset(cpk, 0.0)
            nc.vector.memset(scv[0:64], 1.0)
            nc.vector.memset(scv[64:128], 2.0)
            nc.sync.dma_start(out=wt_fi, in_=wfi)
            nc.sync.dma_start(out=wt_if, in_=wif)
            nc.sync.dma_start(out=wt_og, in_=wog)

            for s in range(S):
                # x for this step -> partition 64, data cols [1, 129)
                nc.sync.dma_start(out=comb[64:65, :, 1 : L + 1], in_=xp[s : s + 1])

                sgate = []   # per-half [f/i or i/f] sigmoid tiles
                sog = []     # per-half [sig(o); sig(2g)] tiles
                tg = tp.tile([128, HB, L], F32)     # tanh(g): [h2 | h1]
                t1 = tp.tile([128, HB, L], F32)     # sf*c:   [h1 | h2]
                t2 = tp.tile([128, HB, L], F32)     # si*tg:  [h1 | h2]
                for half in range(2):
                    s0 = half * HB
                    wgate = wt_fi if half == 0 else wt_if
                    pfi = ps_fi.tile([128, HB, L], F32, name="pfi")
                    pog = ps_og.tile([128, HB, L], F32, name="pog")
                    for chunk in range(4):
                        c0 = chunk * 4
                        for t in range(3):
                            rhs = comb[:, s0 + c0 : s0 + c0 + 4, t : t + L]
                            st, sp_ = (t == 0), (t == 2)
                            nc.tensor.matmul(
                                pfi[:, c0 : c0 + 4, :], lhsT=wgate[:, t, :],
                                rhs=rhs, start=st, stop=sp_,
                            )
                            nc.tensor.matmul(
                                pog[:, c0 : c0 + 4, :], lhsT=wt_og[:, t, :],
                                rhs=rhs, start=st, stop=sp_,
                            )
                    sg = sp.tile([128, HB, L], F32, name="sgate")
                    nc.scalar.activation(sg, pfi, AF.Sigmoid)
                    so = sp.tile([128, HB, L], F32, name="sog")
                    nc.scalar.activation(so, pog, AF.Sigmoid, scale=scv)
                    sgate.append(sg)
                    sog.append(so)
                    # tanh(g) = 2*sig(2g) - 1 ; h1 -> tg[64:], h2 -> tg[:64]
                    dst = tg[64:128] if half == 0 else tg[0:64]
                    nc.vector.tensor_scalar(
                        out=dst, in0=so[64:128], scalar1=2.0, scalar2=-1.0,
                        op0=mybir.AluOpType.mult, op1=mybir.AluOpType.add,
                    )
                # products (gpsimd takes t1 to offload DVE)
                # h1: sf = sgate[0][0:64], c_h1 = cpk[0:64]   -> t1[0:64]
                # h2: sf = sgate[1][64:128], c_h2 = cpk[64:128] -> t1[64:128]
                nc.gpsimd.tensor_mul(t1[0:64], sgate[0][0:64], cpk[0:64])
                nc.gpsimd.tensor_mul(t1[64:128], sgate[1][64:128], cpk[64:128])
                # h1: si = sgate[0][64:128], tg_h1 = tg[64:128] -> t2[0:64]
                # h2: si = sgate[1][0:64],   tg_h2 = tg[0:64]   -> t2[64:128]
                nc.vector.tensor_mul(t2[0:64], sgate[0][64:128], tg[64:128])
                nc.vector.tensor_mul(t2[64:128], sgate[1][0:64], tg[0:64])
                nc.vector.tensor_add(cpk, t1, t2)
                # tanh(c) = 2*sig(2c) - 1
                tcs = tp.tile([128, HB, L], F32, name="tcs")
                nc.scalar.activation(tcs, cpk, AF.Sigmoid, scale=2.0)
                tcf = tp.tile([64, BLOC, L], F32, name="tcf")
                nc.vector.tensor_scalar(
                    out=tcf[:, 0:HB, :], in0=tcs[0:64], scalar1=2.0, scalar2=-1.0,
                    op0=mybir.AluOpType.mult, op1=mybir.AluOpType.add,
                )
                nc.vector.tensor_scalar(
                    out=tcf[:, HB:BLOC, :], in0=tcs[64:128], scalar1=2.0,
                    scalar2=-1.0, op0=mybir.AluOpType.mult, op1=mybir.AluOpType.add,
                )
                # h = sig(o) * tanh(c) -> comb h rows (next step input)
                for half in range(2):
                    s0 = half * HB
                    nc.gpsimd.tensor_mul(
                        comb[0:64, s0 : s0 + HB, 1 : L + 1],
                        sog[half][0:64],
                        tcf[:, s0 : s0 + HB, :],
                    )
                    nc.sync.dma_start(
                        out=hs[s, :, s0 : s0 + HB, :],
                        in_=comb[0:64, s0 : s0 + HB, 1 : L + 1].bitcast(F32),
                    )
    nc.compile()
    return nc


# ---------------------------------------------------------------- stage 2
@lru_cache(maxsize=1)
def _build_stage2():
    nc = bacc.Bacc("TRN2", target_bir_lowering=False, debug=False, num_devices=NC)
    w1p = nc.dram_tensor("w1p", [KCH, 128, 768], F32R, kind="ExternalInput").ap()
    ft = nc.dram_tensor("ft", [KCH, 128, B], F32R, kind="ExternalInput").ap()
    y1p = nc.dram_tensor("y1p", [768, B], F32, kind="ExternalOutput").ap()

    with tile.TileContext(nc) as tc:
        with (
            tc.tile_pool(name="wp", bufs=4) as wp,
            tc.tile_pool(name="rp", bufs=4) as rp,
            tc.tile_pool(name="op", bufs=2) as op,
            tc.tile_pool(name="ps", bufs=1, space="PSUM") as ps,
        ):
            acc = [ps.tile([128, B], F32, name=f"acc{m}") for m in range(6)]
            for k in range(KCH):
                wt = wp.tile([128, 768], F32R, name="wt")
                rt = rp.tile([128, B], F32R, name="rt")
                nc.sync.dma_start(out=wt, in_=w1p[k])
                nc.sync.dma_start(out=rt, in_=ft[k])
                for m in range(6):
                    nc.tensor.matmul(
                        acc[m], lhsT=wt[:, m * 128 : (m + 1) * 128], rhs=rt,
                        start=(k == 0), stop=(k == KCH - 1),
                    )
            for m in range(6):
                ot = op.tile([128, B], F32, name="ot")
                nc.vector.tensor_copy(ot, acc[m])
                nc.sync.dma_start(out=y1p[m * 128 : (m + 1) * 128], in_=ot)
    nc.compile()
    return nc


# ---------------------------------------------------------------- stage 3
@lru_cache(maxsize=1)
def _build_stage3():
    nc = bacc.Bacc("TRN2", target_bir_lowering=False, debug=False, num_devices=NC)
    y1s = nc.dram_tensor("y1s", [6, 128, BLOC], F32R, kind="ExternalInput").ap()
    c1t = nc.dram_tensor("c1t", [128, 6], F32, kind="ExternalInput").ap()
    w2p = nc.dram_tensor("w2p", [6, 128, 12], F32R, kind="ExternalInput").ap()
    c2t = nc.dram_tensor("c2t", [12, 1], F32, kind="ExternalInput").ap()
    w3e = nc.dram_tensor("w3e", [13, 10], F32R, kind="ExternalInput").ap()
    y3p = nc.dram_tensor("y3p", [BLOC, C], F32, kind="ExternalOutput").ap()

    with tile.TileContext(nc) as tc:
        with (
            tc.tile_pool(name="sb", bufs=1) as sb,
            tc.tile_pool(name="ps", bufs=1, space="PSUM") as ps,
        ):
            yt = sb.tile([128, 6, BLOC], F32R)
            c1 = sb.tile([128, 6], F32)
            w2t = sb.tile([128, 6, 12], F32R)
            c2 = sb.tile([12, 1], F32)
            w3t = sb.tile([13, 10], F32R)
            nc.sync.dma_start(out=yt, in_=y1s.rearrange("k p b -> p k b"))
            nc.sync.dma_start(out=c1, in_=c1t)
            nc.sync.dma_start(out=w2t, in_=w2p.rearrange("k p m -> p k m"))
            nc.sync.dma_start(out=c2, in_=c2t)
            nc.sync.dma_start(out=w3t, in_=w3e)

            r1 = sb.tile([128, 6, BLOC], F32R)
            for kc in range(6):
                nc.scalar.activation(
                    r1[:, kc, :], yt[:, kc, :], AF.Relu, bias=c1[:, kc : kc + 1]
                )
            p2 = ps.tile([12, BLOC], F32)
            for kc in range(6):
                nc.tensor.matmul(
                    p2, lhsT=w2t[:, kc, :], rhs=r1[:, kc, :],
                    start=(kc == 0), stop=(kc == 5),
                )
            r2 = sb.tile([13, BLOC], F32R)
            # ones row lives at partition 12 (not 32-aligned): fill the whole
            # tile with 1.0 first, then overwrite rows 0..11 via ACT
            nc.vector.memset(r2.bitcast(F32), 1.0)
            nc.scalar.activation(r2[0:12], p2, AF.Relu, bias=c2)
            p3 = ps.tile([BLOC, C], F32)
            nc.tensor.matmul(p3, lhsT=r2, rhs=w3t, start=True, stop=True)
            ot = sb.tile([BLOC, C], F32)
            nc.vector.tensor_copy(ot, p3)
            nc.sync.dma_start(out=y3p, in_=ot)
    nc.compile()
    return nc


# ---------------------------------------------------------------- host glue
def _prep_stage1_inputs(x, conv_w, conv_b):
    """Build per-core stage-1 in_maps. conv_w: (4H, 1+H, 3); ci order in the
    reference combined tensor is [x, h0..h63]; our comb rows are
    [h0..h63, x, ones]."""
    f32 = np.float32
    ci_perm = np.concatenate([np.arange(1, 65), [0]])       # comb rows 0..64
    w = np.ascontiguousarray(conv_w[:, ci_perm, :], f32)    # (256, 65, 3)
    bi, bf, bo, bg = (conv_b[0:64], conv_b[64:128],
                      conv_b[128:192], conv_b[192:256])
    wi, wf, wo, wg = w[0:64], w[64:128], w[128:192], w[192:256]

    def lhst(wa, wb, ba, bb):
        # -> (66, 3, 128): rows = [65 ci rows, bias row]; cols = [A(64)|B(64)]
        out = np.zeros((66, 3, 128), f32)
        out[0:65, :, 0:64] = wa.transpose(1, 2, 0)
        out[0:65, :, 64:128] = wb.transpose(1, 2, 0)
        out[65, 1, 0:64] = ba
        out[65, 1, 64:128] = bb
        return out

    wfi = lhst(wf, wi, bf, bi)
    wif = lhst(wi, wf, bi, bf)
    wog = lhst(wo, wg, bo, bg)
    maps = []
    for c in range(NC):
        xc = np.ascontiguousarray(
            x[c * BLOC : (c + 1) * BLOC].transpose(1, 0, 2), f32
        )  # (S, BLOC, L)
        maps.append({"xp": xc, "wfi": wfi, "wif": wif, "wog": wog})
    return maps


def kernel(**inputs):
    f32 = np.float32
    x = np.asarray(inputs["x"], f32)
    conv_w = np.asarray(inputs["conv_w"], f32)
    conv_b = np.asarray(inputs["conv_b"], f32)
    w1 = np.asarray(inputs["w1"], f32)
    b1 = np.asarray(inputs["b1"], f32)
    g1, be1 = np.asarray(inputs["g1"], f32), np.asarray(inputs["be1"], f32)
    m1, v1 = np.asarray(inputs["m1"], f32), np.asarray(inputs["v1"], f32)
    w2 = np.asarray(inputs["w2"], f32)
    b2 = np.asarray(inputs["b2"], f32)
    g2, be2 = np.asarray(inputs["g2"], f32), np.asarray(inputs["be2"], f32)
    m2, v2 = np.asarray(inputs["m2"], f32), np.asarray(inputs["v2"], f32)
    w3 = np.asarray(inputs["w3"], f32)
    b3 = np.asarray(inputs["b3"], f32)

    # ---- stage 1: ConvLSTM (batch-parallel)
    nc1 = _build_stage1()
    maps1 = _prep_stage1_inputs(x, conv_w, conv_b)
    res1 = run_bass_kernel_spmd(nc1, maps1, core_ids=CORE_IDS)
    hs_all = np.stack([res1.results[c]["hs"] for c in range(NC)])  # (8,S,H,32,L)

    # ---- reshard: (8,S,H,32,L) -> flatT (S*H*L, 256), feature-major
    flatT = np.ascontiguousarray(
        hs_all.transpose(1, 2, 4, 0, 3), f32
    ).reshape(KTOT, B)

    # ---- stage 2: big GEMM, contract-dim sharded
    s1 = g1 / np.sqrt(v1 + EPS)
    c1 = b1 * s1 + (be1 - m1 * s1)
    w1sT = np.ascontiguousarray((w1 * s1[:, None]).T, f32)         # (KTOT, 768)
    nc2 = _build_stage2()
    maps2 = []
    for c in range(NC):
        sl = slice(c * KSH, (c + 1) * KSH)
        maps2.append({
            "w1p": np.ascontiguousarray(w1sT[sl]).reshape(KCH, 128, 768),
            "ft": np.ascontiguousarray(flatT[sl]).reshape(KCH, 128, B),
        })
    res2 = run_bass_kernel_spmd(nc2, maps2, core_ids=CORE_IDS)
    y1 = np.sum([res2.results[c]["y1p"] for c in range(NC)], axis=0,
                dtype=np.float64).astype(f32)                       # (768, 256)

    # ---- stage 3: epilogue (batch-parallel)
    s2 = g2 / np.sqrt(v2 + EPS)
    c2 = b2 * s2 + (be2 - m2 * s2)
    c1t = np.ascontiguousarray(c1.reshape(6, 128).T, f32)           # (128, 6)
    w2p = np.ascontiguousarray(
        (w2 * s2[:, None]).T.reshape(6, 128, 12), f32
    )
    w3e = np.concatenate([w3.T, b3[None, :]], axis=0).astype(f32)   # (13, 10)
    nc3 = _build_stage3()
    maps3 = []
    for c in range(NC):
        ysl = np.ascontiguousarray(
            y1[:, c * BLOC : (c + 1) * BLOC]
        ).reshape(6, 128, BLOC)
        maps3.append({
            "y1s": ysl, "c1t": c1t, "w2p": w2p,
            "c2t": c2.reshape(12, 1).astype(f32), "w3e": w3e,
        })
    res3 = run_bass_kernel_spmd(nc3, maps3, core_ids=CORE_IDS)
    y3 = np.concatenate([res3.results[c]["y3p"] for c in range(NC)], axis=0)
    return np.ascontiguousarray(y3, f32)


# revision 4
# speedup vs baseline: 26890.6974x; 26890.6974x over previous
"""Trainium2 Bass kernel for nn_CropConvLSTM.

Model: ConvLSTM (Conv1d(1+H -> 4H, k=3, pad=1), S=12 steps) over x (B=256,
S=12, L=128), then head Linear(98304->768)+BN+ReLU, Linear(768->12)+BN+ReLU,
Linear(12->10).

Distribution over 8 NeuronCores, three launches:
  Stage 1: ConvLSTM, data-parallel over batch (32 samples/core). Conv done as
    3 shifted fp32r matmuls (K=66: 64 h-channels + x-row + ones-row for the
    conv bias) accumulating in PSUM; gates on ACT/DVE/GPSIMD with all tensors
    at legal partition bases (tanh computed as 2*sigmoid(2x)-1 so a single
    per-partition-scaled sigmoid covers the [o;g] psum tile).
  Stage 2: y1 = flat @ (w1*bn1_scale).T, sharded over the 98304 contract dim
    (12288 features/core); each core emits a partial (768, 256), host reduces.
  Stage 3: bias+ReLU, Linear2+BN+ReLU, Linear3 (+b3 via ones-row trick),
    data-parallel over batch again.

BN (eval mode) is folded into the weights/biases on the host.
"""
import os
import sys

sys.path.insert(0, "/opt/trn_rl_repo")

from functools import lru_cache

import numpy as np

import concourse.bass as bass
import concourse.tile as tile
from concourse import bacc, mybir
from concourse.bass_utils import run_bass_kernel_spmd

F32 = mybir.dt.float32
F32R = mybir.dt.float32r
AF = mybir.ActivationFunctionType

B, S, L, H, C = 256, 12, 128, 64, 10
NC = 8
BLOC = B // NC            # 32 samples per core in stages 1/3
KTOT = S * H * L          # 98304
KSH = KTOT // NC          # 12288 contract features per core in stage 2
KCH = KSH // 128          # 96 k-chunks per core
EPS = 1e-5
CORE_IDS = list(range(NC))


# ---------------------------------------------------------------- stage 1
@lru_cache(maxsize=1)
def _build_stage1():
    nc = bacc.Bacc("TRN2", target_bir_lowering=False, debug=False, num_devices=NC)
    xp = nc.dram_tensor("xp", [S, BLOC, L], F32R, kind="ExternalInput").ap()
    wfi = nc.dram_tensor("wfi", [66, 3, 128], F32R, kind="ExternalInput").ap()
    wif = nc.dram_tensor("wif", [66, 3, 128], F32R, kind="ExternalInput").ap()
    wog = nc.dram_tensor("wog", [66, 3, 128], F32R, kind="ExternalInput").ap()
    hs = nc.dram_tensor("hs", [S, H, BLOC, L], F32, kind="ExternalOutput").ap()

    HB = BLOC // 2        # 16 samples per half-step
    FH = HB * L           # 2048 free per half

    with tile.TileContext(nc) as tc:
        with (
            tc.tile_pool(name="persist", bufs=1) as pp,
            tc.tile_pool(name="sig", bufs=2) as sp,
            tc.tile_pool(name="tmp", bufs=2) as tp,
            tc.tile_pool(name="psfi", bufs=1, space="PSUM") as ps_fi,
            tc.tile_pool(name="psog", bufs=1, space="PSUM") as ps_og,
        ):
            comb = pp.tile([66, BLOC, L + 2], F32R)     # [h(64); x; ones]
            cpk = pp.tile([128, HB, L], F32)            # c: [h1 | h2] packed
            wt_fi = pp.tile([66, 3, 128], F32R)
            wt_if = pp.tile([66, 3, 128], F32R)
            wt_og = pp.tile([66, 3, 128], F32R)
            scv = pp.tile([128, 1], F32)                # act scale [1;2]

            nc.vector.memset(comb.bitcast(F32), 0.0)
            # ones row is partition 65; engine bases must be 32-aligned, so
            # set [64:66] to 1.0 then zero the x row (64) again
            nc.vector.memset(comb[64:66].bitcast(F32), 1.0)
            nc.vector.memset(comb[64:65].bitcast(F32), 0.0)
            nc.vector.memset(cpk, 0.0)
            nc.vector.memset(scv[0:64], 1.0)
            nc.vector.memset(scv[64:128], 2.0)
            nc.sync.dma_start(out=wt_fi, in_=wfi)
            nc.sync.dma_start(out=wt_if, in_=wif)
            nc.sync.dma_start(out=wt_og, in_=wog)

            for s in range(S):
                # x for this step -> partition 64, data cols [1, 129)
                nc.sync.dma_start(out=comb[64:65, :, 1 : L + 1], in_=xp[s : s + 1])

                sgate = []   # per-half [f/i or i/f] sigmoid tiles
                sog = []     # per-half [sig(o); sig(2g)] tiles
                tg = tp.tile([128, HB, L], F32)     # tanh(g): [h2 | h1]
                t1 = tp.tile([128, HB, L], F32)     # sf*c:   [h1 | h2]
                t2 = tp.tile([128, HB, L], F32)     # si*tg:  [h1 | h2]
                for half in range(2):
                    s0 = half * HB
                    wgate = wt_fi if half == 0 else wt_if
                    pfi = ps_fi.tile([128, HB, L], F32, name="pfi")
                    pog = ps_og.tile([128, HB, L], F32, name="pog")
                    for chunk in range(4):
                        c0 = chunk * 4
                        for t in range(3):
                            rhs = comb[:, s0 + c0 : s0 + c0 + 4, t : t + L]
                            st, sp_ = (t == 0), (t == 2)
                            nc.tensor.matmul(
                                pfi[:, c0 : c0 + 4, :], lhsT=wgate[:, t, :],
                                rhs=rhs, start=st, stop=sp_,
                            )
                            nc.tensor.matmul(
                                pog[:, c0 : c0 + 4, :], lhsT=wt_og[:, t, :],
                                rhs=rhs, start=st, stop=sp_,
                            )
                    sg = sp.tile([128, HB, L], F32, name="sgate")
                    nc.scalar.activation(sg, pfi, AF.Sigmoid)
                    so = sp.tile([128, HB, L], F32, name="sog")
                    nc.scalar.activation(so, pog, AF.Sigmoid, scale=scv)
                    sgate.append(sg)
                    sog.append(so)
                    # tanh(g) = 2*sig(2g) - 1 ; h1 -> tg[64:], h2 -> tg[:64]
                    dst = tg[64:128] if half == 0 else tg[0:64]
                    nc.vector.tensor_scalar(
                        out=dst, in0=so[64:128], scalar1=2.0, scalar2=-1.0,
                        op0=mybir.AluOpType.mult, op1=mybir.AluOpType.add,
                    )
                # products (gpsimd takes t1 to offload DVE)
                # h1: sf = sgate[0][0:64], c_h1 = cpk[0:64]   -> t1[0:64]
                # h2: sf = sgate[1][64:128], c_h2 = cpk[64:128] -> t1[64:128]
                nc.gpsimd.tensor_mul(t1[0:64], sgate[0][0:64], cpk[0:64])
                nc.gpsimd.tensor_mul(t1[64:128], sgate[1][64:128], cpk[64:128])
                # h1: si = sgate[0][64:128], tg_h1 = tg[64:128] -> t2[0:64]
                # h2: si = sgate[1][0:64],   tg_h2 = tg[0:64]   -> t2[64:128]
                nc.vector.tensor_mul(t2[0:64], sgate[0][64:128], tg[64:128])
                nc.vector.tensor_mul(t2[64:128], sgate[1][0:64], tg[0:64])
                nc.vector.tensor_add(cpk, t1, t2)
                # tanh(c) = 2*sig(2c) - 1
                tcs = tp.tile([128, HB, L], F32, name="tcs")
                nc.scalar.activation(tcs, cpk, AF.Sigmoid, scale=2.0)
                tcf = tp.tile([64, BLOC, L], F32, name="tcf")
                nc.vector.tensor_scalar(
                    out=tcf[:, 0:HB, :], in0=tcs[0:64], scalar1=2.0, scalar2=-1.0,
                    op0=mybir.AluOpType.mult, op1=mybir.AluOpType.add,
                )
                nc.vector.tensor_scalar(
                    out=tcf[:, HB:BLOC, :], in0=tcs[64:128], scalar1=2.0,
                    scalar2=-1.0, op0=mybir.AluOpType.mult, op1=mybir.AluOpType.add,
                )
                # h = sig(o) * tanh(c) -> comb h rows (next step input)
                for half in range(2):
                    s0 = half * HB
                    nc.gpsimd.tensor_mul(
                        comb[0:64, s0 : s0 + HB, 1 : L + 1],
                        sog[half][0:64],
                        tcf[:, s0 : s0 + HB, :],
                    )
                    nc.sync.dma_start(
                        out=hs[s, :, s0 : s0 + HB, :],
                        in_=comb[0:64, s0 : s0 + HB, 1 : L + 1].bitcast(F32),
                    )
    nc.compile()
    return nc


# ---------------------------------------------------------------- stage 2
@lru_cache(maxsize=1)
def _build_stage2():
    nc = bacc.Bacc("TRN2", target_bir_lowering=False, debug=False, num_devices=NC)
    w1p = nc.dram_tensor("w1p", [KCH, 128, 768], F32R, kind="ExternalInput").ap()
    ft = nc.dram_tensor("ft", [KCH, 128, B], F32R, kind="ExternalInput").ap()
    y1p = nc.dram_tensor("y1p", [768, B], F32, kind="ExternalOutput").ap()

    with tile.TileContext(nc) as tc:
        with (
            tc.tile_pool(name="wp", bufs=4) as wp,
            tc.tile_pool(name="rp", bufs=4) as rp,
            tc.tile_pool(name="op", bufs=2) as op,
            tc.tile_pool(name="ps", bufs=1, space="PSUM") as ps,
        ):
            acc = [ps.tile([128, B], F32, name=f"acc{m}") for m in range(6)]
            for k in range(KCH):
                wt = wp.tile([128, 768], F32R, name="wt")
                rt = rp.tile([128, B], F32R, name="rt")
                nc.sync.dma_start(out=wt, in_=w1p[k])
                nc.sync.dma_start(out=rt, in_=ft[k])
                for m in range(6):
                    nc.tensor.matmul(
                        acc[m], lhsT=wt[:, m * 128 : (m + 1) * 128], rhs=rt,
                        start=(k == 0), stop=(k == KCH - 1),
                    )
            for m in range(6):
                ot = op.tile([128, B], F32, name="ot")
                nc.vector.tensor_copy(ot, acc[m])
                nc.sync.dma_start(out=y1p[m * 128 : (m + 1) * 128], in_=ot)
    nc.compile()
    return nc


# ---------------------------------------------------------------- stage 3
@lru_cache(maxsize=1)
def _build_stage3():
    nc = bacc.Bacc("TRN2", target_bir_lowering=False, debug=False, num_devices=NC)
    y1s = nc.dram_tensor("y1s", [6, 128, BLOC], F32R, kind="ExternalInput").ap()
    c1t = nc.dram_tensor("c1t", [128, 6], F32, kind="ExternalInput").ap()
    w2p = nc.dram_tensor("w2p", [6, 128, 12], F32R, kind="ExternalInput").ap()
    c2t = nc.dram_tensor("c2t", [12, 1], F32, kind="ExternalInput").ap()
    w3e = nc.dram_tensor("w3e", [13, 10], F32R, kind="ExternalInput").ap()
    y3p = nc.dram_tensor("y3p", [BLOC, C], F32, kind="ExternalOutput").ap()

    with tile.TileContext(nc) as tc:
        with (
            tc.tile_pool(name="sb", bufs=1) as sb,
            tc.tile_pool(name="ps", bufs=1, space="PSUM") as ps,
        ):
            yt = sb.tile([128, 6, BLOC], F32R)
            c1 = sb.tile([128, 6], F32)
            w2t = sb.tile([128, 6, 12], F32R)
            c2 = sb.tile([12, 1], F32)
            w3t = sb.tile([13, 10], F32R)
            nc.sync.dma_start(out=yt, in_=y1s.rearrange("k p b -> p k b"))
            nc.sync.dma_start(out=c1, in_=c1t)
            nc.sync.dma_start(out=w2t, in_=w2p.rearrange("k p m -> p k m"))
            nc.sync.dma_start(out=c2, in_=c2t)
            nc.sync.dma_start(out=w3t, in_=w3e)

            r1 = sb.tile([128, 6, BLOC], F32R)
            for kc in range(6):
                nc.scalar.activation(
                    r1[:, kc, :], yt[:, kc, :], AF.Relu, bias=c1[:, kc : kc + 1]
                )
            p2 = ps.tile([12, BLOC], F32)
            for kc in range(6):
                nc.tensor.matmul(
                    p2, lhsT=w2t[:, kc, :], rhs=r1[:, kc, :],
                    start=(kc == 0), stop=(kc == 5),
                )
            r2 = sb.tile([13, BLOC], F32R)
            # ones row lives at partition 12 (not 32-aligned): fill the whole
            # tile with 1.0 first, then overwrite rows 0..11 via ACT
            nc.vector.memset(r2.bitcast(F32), 1.0)
            nc.scalar.activation(r2[0:12], p2, AF.Relu, bias=c2)
            p3 = ps.tile([BLOC, C], F32)
            nc.tensor.matmul(p3, lhsT=r2, rhs=w3t, start=True, stop=True)
            ot = sb.tile([BLOC, C], F32)
            nc.vector.tensor_copy(ot, p3)
            nc.sync.dma_start(out=y3p, in_=ot)
    nc.compile()
    return nc


# ---------------------------------------------------------------- host glue
def _prep_stage1_inputs(x, conv_w, conv_b):
    """Build per-core stage-1 in_maps. conv_w: (4H, 1+H, 3); ci order in the
    reference combined tensor is [x, h0..h63]; our comb rows are
    [h0..h63, x, ones]."""
    f32 = np.float32
    ci_perm = np.concatenate([np.arange(1, 65), [0]])       # comb rows 0..64
    w = np.ascontiguousarray(conv_w[:, ci_perm, :], f32)    # (256, 65, 3)
    bi, bf, bo, bg = (conv_b[0:64], conv_b[64:128],
                      conv_b[128:192], conv_b[192:256])
    wi, wf, wo, wg = w[0:64], w[64:128], w[128:192], w[192:256]

    def lhst(wa, wb, ba, bb):
        # -> (66, 3, 128): rows = [65 ci rows, bias row]; cols = [A(64)|B(64)]
        out = np.zeros((66, 3, 128), f32)
        out[0:65, :, 0:64] = wa.transpose(1, 2, 0)
        out[0:65, :, 64:128] = wb.transpose(1, 2, 0)
        out[65, 1, 0:64] = ba
        out[65, 1, 64:128] = bb
        return out

    wfi = lhst(wf, wi, bf, bi)
    wif = lhst(wi, wf, bi, bf)
    wog = lhst(wo, wg, bo, bg)
    maps = []
    for c in range(NC):
        xc = np.ascontiguousarray(
            x[c * BLOC : (c + 1) * BLOC].transpose(1, 0, 2), f32
        )  # (S, BLOC, L)
        maps.append({"xp": xc, "wfi": wfi, "wif": wif, "wog": wog})
    return maps


last_hw_ns = None
last_stage_ns = None


def _run(nc, maps, label):
    trace = bool(int(os.environ.get("BASSK_TRACE", "0")))
    res = run_bass_kernel_spmd(nc, maps, core_ids=CORE_IDS, trace=trace)
    if trace:
        global last_stage_ns
        if last_stage_ns is None:
            last_stage_ns = {}
        last_stage_ns[label] = res.exec_time_ns
    return res


def kernel(**inputs):
    global last_hw_ns, last_stage_ns
    last_stage_ns = None
    f32 = np.float32
    x = np.asarray(inputs["x"], f32)
    conv_w = np.asarray(inputs["conv_w"], f32)
    conv_b = np.asarray(inputs["conv_b"], f32)
    w1 = np.asarray(inputs["w1"], f32)
    b1 = np.asarray(inputs["b1"], f32)
    g1, be1 = np.asarray(inputs["g1"], f32), np.asarray(inputs["be1"], f32)
    m1, v1 = np.asarray(inputs["m1"], f32), np.asarray(inputs["v1"], f32)
    w2 = np.asarray(inputs["w2"], f32)
    b2 = np.asarray(inputs["b2"], f32)
    g2, be2 = np.asarray(inputs["g2"], f32), np.asarray(inputs["be2"], f32)
    m2, v2 = np.asarray(inputs["m2"], f32), np.asarray(inputs["v2"], f32)
    w3 = np.asarray(inputs["w3"], f32)
    b3 = np.asarray(inputs["b3"], f32)

    # ---- stage 1: ConvLSTM (batch-parallel)
    nc1 = _build_stage1()
    maps1 = _prep_stage1_inputs(x, conv_w, conv_b)
    res1 = _run(nc1, maps1, "stage1")
    hs_all = np.stack([res1.results[c]["hs"] for c in range(NC)])  # (8,S,H,32,L)

    # ---- reshard: (8,S,H,32,L) -> flatT (S*H*L, 256), feature-major
    flatT = np.ascontiguousarray(
        hs_all.transpose(1, 2, 4, 0, 3), f32
    ).reshape(KTOT, B)

    # ---- stage 2: big GEMM, contract-dim sharded
    s1 = g1 / np.sqrt(v1 + EPS)
    c1 = b1 * s1 + (be1 - m1 * s1)
    w1sT = np.ascontiguousarray((w1 * s1[:, None]).T, f32)         # (KTOT, 768)
    nc2 = _build_stage2()
    maps2 = []
    for c in range(NC):
        sl = slice(c * KSH, (c + 1) * KSH)
        maps2.append({
            "w1p": np.ascontiguousarray(w1sT[sl]).reshape(KCH, 128, 768),
            "ft": np.ascontiguousarray(flatT[sl]).reshape(KCH, 128, B),
        })
    res2 = _run(nc2, maps2, "stage2")
    y1 = np.sum([res2.results[c]["y1p"] for c in range(NC)], axis=0,
                dtype=np.float64).astype(f32)                       # (768, 256)

    # ---- stage 3: epilogue (batch-parallel)
    s2 = g2 / np.sqrt(v2 + EPS)
    c2 = b2 * s2 + (be2 - m2 * s2)
    c1t = np.ascontiguousarray(c1.reshape(6, 128).T, f32)           # (128, 6)
    w2p = np.ascontiguousarray(
        (w2 * s2[:, None]).T.reshape(6, 128, 12), f32
    )
    w3e = np.concatenate([w3.T, b3[None, :]], axis=0).astype(f32)   # (13, 10)
    nc3 = _build_stage3()
    maps3 = []
    for c in range(NC):
        ysl = np.ascontiguousarray(
            y1[:, c * BLOC : (c + 1) * BLOC]
        ).reshape(6, 128, BLOC)
        maps3.append({
            "y1s": ysl, "c1t": c1t, "w2p": w2p,
            "c2t": c2.reshape(12, 1).astype(f32), "w3e": w3e,
        })
    res3 = _run(nc3, maps3, "stage3")
    y3 = np.concatenate([res3.results[c]["y3p"] for c in range(NC)], axis=0)
    if last_stage_ns and all(v is not None for v in last_stage_ns.values()):
        last_hw_ns = sum(last_stage_ns.values())
    return np.ascontiguousarray(y3, f32)


# revision 6
# speedup vs baseline: 31791.9803x; 1.1823x over previous
"""Trainium2 Bass kernel for nn_CropConvLSTM.

Model: ConvLSTM (Conv1d(1+H -> 4H, k=3, pad=1), S=12 steps) over x (B=256,
S=12, L=128), then head Linear(98304->768)+BN+ReLU, Linear(768->12)+BN+ReLU,
Linear(12->10).

Distribution over 8 NeuronCores, three launches:
  Stage 1: ConvLSTM, data-parallel over batch (32 samples/core). Conv done as
    3 shifted fp32r matmuls (K=66: 64 h-channels + x-row + ones-row for the
    conv bias) accumulating in PSUM; gates on ACT/DVE/GPSIMD with all tensors
    at legal partition bases (tanh computed as 2*sigmoid(2x)-1 so a single
    per-partition-scaled sigmoid covers the [o;g] psum tile).
  Stage 2: y1 = flat @ (w1*bn1_scale).T, sharded over the 98304 contract dim
    (12288 features/core); each core emits a partial (768, 256), host reduces.
  Stage 3: bias+ReLU, Linear2+BN+ReLU, Linear3 (+b3 via ones-row trick),
    data-parallel over batch again.

BN (eval mode) is folded into the weights/biases on the host.
"""
import os
import sys

sys.path.insert(0, "/opt/trn_rl_repo")

from functools import lru_cache

import numpy as np

import concourse.bass as bass
import concourse.tile as tile
from concourse import bacc, mybir
from concourse.bass_utils import run_bass_kernel_spmd

F32 = mybir.dt.float32
F32R = mybir.dt.float32r
BF16 = mybir.dt.bfloat16
AF = mybir.ActivationFunctionType

B, S, L, H, C = 256, 12, 128, 64, 10
NC = 8
BLOC = B // NC            # 32 samples per core in stages 1/3
KTOT = S * H * L          # 98304
KSH = KTOT // NC          # 12288 contract features per core in stage 2
KCH = KSH // 128          # 96 k-chunks per core
EPS = 1e-5
CORE_IDS = list(range(NC))


# ---------------------------------------------------------------- stage 1
@lru_cache(maxsize=1)
def _build_stage1():
    nc = bacc.Bacc("TRN2", target_bir_lowering=False, debug=False, num_devices=NC)
    xp = nc.dram_tensor("xp", [S, BLOC, L], BF16, kind="ExternalInput").ap()
    wfi = nc.dram_tensor("wfi", [66, 3, 128], BF16, kind="ExternalInput").ap()
    wif = nc.dram_tensor("wif", [66, 3, 128], BF16, kind="ExternalInput").ap()
    wog = nc.dram_tensor("wog", [66, 3, 128], BF16, kind="ExternalInput").ap()
    hs = nc.dram_tensor("hs", [S, H, BLOC, L], BF16, kind="ExternalOutput").ap()

    HB = BLOC // 2        # 16 samples per half-step

    with tile.TileContext(nc) as tc:
        with (
            tc.tile_pool(name="persist", bufs=1) as pp,
            tc.tile_pool(name="sig", bufs=2) as sp,
            tc.tile_pool(name="tmp", bufs=2) as tp,
            tc.tile_pool(name="psfi", bufs=1, space="PSUM") as ps_fi,
            tc.tile_pool(name="psog", bufs=1, space="PSUM") as ps_og,
        ):
            comb = pp.tile([66, BLOC, L + 2], BF16)     # [h(64); x; ones]
            cpk = pp.tile([128, HB, L], BF16)           # c: [h1 | h2] packed
            wt_fi = pp.tile([66, 3, 128], BF16)
            wt_if = pp.tile([66, 3, 128], BF16)
            wt_og = pp.tile([66, 3, 128], BF16)
            scv = pp.tile([128, 1], F32)                # act scale [1;2]

            nc.vector.memset(comb, 0.0)
            # ones row is partition 65; engine bases must be 32-aligned, so
            # set [64:66] to 1.0 then zero the x row (64) again
            nc.vector.memset(comb[64:66], 1.0)
            nc.vector.memset(comb[64:65], 0.0)
            nc.vector.memset(cpk, 0.0)
            nc.vector.memset(scv[0:64], 1.0)
            nc.vector.memset(scv[64:128], 2.0)
            nc.sync.dma_start(out=wt_fi, in_=wfi)
            nc.sync.dma_start(out=wt_if, in_=wif)
            nc.sync.dma_start(out=wt_og, in_=wog)

            for s in range(S):
                # x for this step -> partition 64, data cols [1, 129)
                nc.sync.dma_start(out=comb[64:65, :, 1 : L + 1], in_=xp[s : s + 1])

                sgate = []   # per-half [f/i or i/f] sigmoid tiles
                sog = []     # per-half [sig(o); sig(2g)] tiles
                tg = tp.tile([128, HB, L], BF16)    # tanh(g): [h2 | h1]
                t1 = tp.tile([128, HB, L], BF16)    # sf*c:   [h1 | h2]
                t2 = tp.tile([128, HB, L], BF16)    # si*tg:  [h1 | h2]
                for half in range(2):
                    s0 = half * HB
                    wgate = wt_fi if half == 0 else wt_if
                    pfi = ps_fi.tile([128, HB, L], F32, name="pfi")
                    pog = ps_og.tile([128, HB, L], F32, name="pog")
                    for chunk in range(4):
                        c0 = chunk * 4
                        for t in range(3):
                            rhs = comb[:, s0 + c0 : s0 + c0 + 4, t : t + L]
                            st, sp_ = (t == 0), (t == 2)
                            nc.tensor.matmul(
                                pfi[:, c0 : c0 + 4, :], lhsT=wgate[:, t, :],
                                rhs=rhs, start=st, stop=sp_,
                            )
                            nc.tensor.matmul(
                                pog[:, c0 : c0 + 4, :], lhsT=wt_og[:, t, :],
                                rhs=rhs, start=st, stop=sp_,
                            )
                    sg = sp.tile([128, HB, L], BF16, name="sgate")
                    nc.scalar.activation(sg, pfi, AF.Sigmoid)
                    # keep [sig(o); sig(2g)] in fp32: the 2x-1 unfold would
                    # amplify bf16 rounding of sig into absolute tanh error
                    so = sp.tile([128, HB, L], F32, name="sog")
                    nc.scalar.activation(so, pog, AF.Sigmoid, scale=scv)
                    sob = sp.tile([64, HB, L], BF16, name=f"sob{half}")
                    nc.vector.tensor_copy(sob, so[0:64])
                    sgate.append(sg)
                    sog.append(sob)
                    # tanh(g) = 2*sig(2g) - 1 ; h1 -> tg[64:], h2 -> tg[:64]
                    dst = tg[64:128] if half == 0 else tg[0:64]
                    nc.vector.tensor_scalar(
                        out=dst, in0=so[64:128], scalar1=2.0, scalar2=-1.0,
                        op0=mybir.AluOpType.mult, op1=mybir.AluOpType.add,
                    )
                # h1: sf = sgate[0][0:64], c_h1 = cpk[0:64]   -> t1[0:64]
                # h2: sf = sgate[1][64:128], c_h2 = cpk[64:128] -> t1[64:128]
                nc.gpsimd.tensor_mul(t1[0:64], sgate[0][0:64], cpk[0:64])
                nc.gpsimd.tensor_mul(t1[64:128], sgate[1][64:128], cpk[64:128])
                # h1: si = sgate[0][64:128], tg_h1 = tg[64:128] -> t2[0:64]
                # h2: si = sgate[1][0:64],   tg_h2 = tg[0:64]   -> t2[64:128]
                nc.vector.tensor_mul(t2[0:64], sgate[0][64:128], tg[64:128])
                nc.vector.tensor_mul(t2[64:128], sgate[1][0:64], tg[0:64])
                nc.vector.tensor_add(cpk, t1, t2)
                # tanh(c) = 2*sig(2c) - 1 (fp32 intermediate, see above)
                tcs = tp.tile([128, HB, L], F32, name="tcs")
                nc.scalar.activation(tcs, cpk, AF.Sigmoid, scale=2.0)
                tcf = tp.tile([64, BLOC, L], BF16, name="tcf")
                nc.vector.tensor_scalar(
                    out=tcf[:, 0:HB, :], in0=tcs[0:64], scalar1=2.0, scalar2=-1.0,
                    op0=mybir.AluOpType.mult, op1=mybir.AluOpType.add,
                )
                nc.vector.tensor_scalar(
                    out=tcf[:, HB:BLOC, :], in0=tcs[64:128], scalar1=2.0,
                    scalar2=-1.0, op0=mybir.AluOpType.mult, op1=mybir.AluOpType.add,
                )
                # h = sig(o) * tanh(c) -> comb h rows (next step input)
                for half in range(2):
                    s0 = half * HB
                    nc.gpsimd.tensor_mul(
                        comb[0:64, s0 : s0 + HB, 1 : L + 1],
                        sog[half],
                        tcf[:, s0 : s0 + HB, :],
                    )
                    nc.sync.dma_start(
                        out=hs[s, :, s0 : s0 + HB, :],
                        in_=comb[0:64, s0 : s0 + HB, 1 : L + 1],
                    )
    nc.compile()
    return nc


# ---------------------------------------------------------------- stage 2
@lru_cache(maxsize=1)
def _build_stage2():
    nc = bacc.Bacc("TRN2", target_bir_lowering=False, debug=False, num_devices=NC)
    w1p = nc.dram_tensor("w1p", [KCH, 128, 768], BF16, kind="ExternalInput").ap()
    ft = nc.dram_tensor("ft", [KCH, 128, B], BF16, kind="ExternalInput").ap()
    y1p = nc.dram_tensor("y1p", [768, B], F32, kind="ExternalOutput").ap()

    KB = 4                       # k-chunks per DMA batch
    NB = KCH // KB               # 24 batches

    with tile.TileContext(nc) as tc:
        with (
            tc.tile_pool(name="wp", bufs=3) as wp,
            tc.tile_pool(name="rp", bufs=3) as rp,
            tc.tile_pool(name="op", bufs=2) as op,
            tc.tile_pool(name="ps", bufs=1, space="PSUM") as ps,
        ):
            acc = [ps.tile([128, B], F32, name=f"acc{m}") for m in range(6)]
            for kb in range(NB):
                wt = wp.tile([128, KB, 768], BF16, name="wt")
                rt = rp.tile([128, KB, B], BF16, name="rt")
                nc.sync.dma_start(
                    out=wt,
                    in_=w1p[kb * KB : (kb + 1) * KB].rearrange("k p m -> p k m"),
                )
                nc.gpsimd.dma_start(
                    out=rt,
                    in_=ft[kb * KB : (kb + 1) * KB].rearrange("k p b -> p k b"),
                )
                for kc in range(KB):
                    for m in range(6):
                        nc.tensor.matmul(
                            acc[m], lhsT=wt[:, kc, m * 128 : (m + 1) * 128],
                            rhs=rt[:, kc, :],
                            start=(kb == 0 and kc == 0),
                            stop=(kb == NB - 1 and kc == KB - 1),
                        )
            for m in range(6):
                ot = op.tile([128, B], F32, name="ot")
                nc.vector.tensor_copy(ot, acc[m])
                nc.sync.dma_start(out=y1p[m * 128 : (m + 1) * 128], in_=ot)
    nc.compile()
    return nc


# ---------------------------------------------------------------- stage 3
@lru_cache(maxsize=1)
def _build_stage3():
    nc = bacc.Bacc("TRN2", target_bir_lowering=False, debug=False, num_devices=NC)
    y1s = nc.dram_tensor("y1s", [6, 128, BLOC], F32R, kind="ExternalInput").ap()
    c1t = nc.dram_tensor("c1t", [128, 6], F32, kind="ExternalInput").ap()
    w2p = nc.dram_tensor("w2p", [6, 128, 12], F32R, kind="ExternalInput").ap()
    c2t = nc.dram_tensor("c2t", [12, 1], F32, kind="ExternalInput").ap()
    w3e = nc.dram_tensor("w3e", [13, 10], F32R, kind="ExternalInput").ap()
    y3p = nc.dram_tensor("y3p", [BLOC, C], F32, kind="ExternalOutput").ap()

    with tile.TileContext(nc) as tc:
        with (
            tc.tile_pool(name="sb", bufs=1) as sb,
            tc.tile_pool(name="ps", bufs=1, space="PSUM") as ps,
        ):
            yt = sb.tile([128, 6, BLOC], F32R)
            c1 = sb.tile([128, 6], F32)
            w2t = sb.tile([128, 6, 12], F32R)
            c2 = sb.tile([12, 1], F32)
            w3t = sb.tile([13, 10], F32R)
            nc.sync.dma_start(out=yt, in_=y1s.rearrange("k p b -> p k b"))
            nc.sync.dma_start(out=c1, in_=c1t)
            nc.sync.dma_start(out=w2t, in_=w2p.rearrange("k p m -> p k m"))
            nc.sync.dma_start(out=c2, in_=c2t)
            nc.sync.dma_start(out=w3t, in_=w3e)

            r1 = sb.tile([128, 6, BLOC], F32R)
            for kc in range(6):
                nc.scalar.activation(
                    r1[:, kc, :], yt[:, kc, :], AF.Relu, bias=c1[:, kc : kc + 1]
                )
            p2 = ps.tile([12, BLOC], F32)
            for kc in range(6):
                nc.tensor.matmul(
                    p2, lhsT=w2t[:, kc, :], rhs=r1[:, kc, :],
                    start=(kc == 0), stop=(kc == 5),
                )
            r2 = sb.tile([13, BLOC], F32R)
            # ones row lives at partition 12 (not 32-aligned): fill the whole
            # tile with 1.0 first, then overwrite rows 0..11 via ACT
            nc.vector.memset(r2.bitcast(F32), 1.0)
            nc.scalar.activation(r2[0:12], p2, AF.Relu, bias=c2)
            p3 = ps.tile([BLOC, C], F32)
            nc.tensor.matmul(p3, lhsT=r2, rhs=w3t, start=True, stop=True)
            ot = sb.tile([BLOC, C], F32)
            nc.vector.tensor_copy(ot, p3)
            nc.sync.dma_start(out=y3p, in_=ot)
    nc.compile()
    return nc


# ---------------------------------------------------------------- host glue
def _prep_stage1_inputs(x, conv_w, conv_b):
    """Build per-core stage-1 in_maps. conv_w: (4H, 1+H, 3); ci order in the
    reference combined tensor is [x, h0..h63]; our comb rows are
    [h0..h63, x, ones]."""
    import ml_dtypes
    bf = ml_dtypes.bfloat16
    f32 = np.float32
    ci_perm = np.concatenate([np.arange(1, 65), [0]])       # comb rows 0..64
    w = np.ascontiguousarray(conv_w[:, ci_perm, :], f32)    # (256, 65, 3)
    bi, bf_, bo, bg = (conv_b[0:64], conv_b[64:128],
                       conv_b[128:192], conv_b[192:256])
    wi, wf, wo, wg = w[0:64], w[64:128], w[128:192], w[192:256]

    def lhst(wa, wb, ba, bb):
        # -> (66, 3, 128): rows = [65 ci rows, bias row]; cols = [A(64)|B(64)]
        out = np.zeros((66, 3, 128), f32)
        out[0:65, :, 0:64] = wa.transpose(1, 2, 0)
        out[0:65, :, 64:128] = wb.transpose(1, 2, 0)
        out[65, 1, 0:64] = ba
        out[65, 1, 64:128] = bb
        return out

    wfi = lhst(wf, wi, bf_, bi).astype(bf)
    wif = lhst(wi, wf, bi, bf_).astype(bf)
    wog = lhst(wo, wg, bo, bg).astype(bf)
    maps = []
    for c in range(NC):
        xc = np.ascontiguousarray(
            x[c * BLOC : (c + 1) * BLOC].transpose(1, 0, 2)
        ).astype(bf)  # (S, BLOC, L)
        maps.append({"xp": xc, "wfi": wfi, "wif": wif, "wog": wog})
    return maps


last_hw_ns = None
last_stage_ns = None


def _run(nc, maps, label):
    trace = bool(int(os.environ.get("BASSK_TRACE", "0")))
    res = run_bass_kernel_spmd(nc, maps, core_ids=CORE_IDS, trace=trace)
    if trace:
        global last_stage_ns
        if last_stage_ns is None:
            last_stage_ns = {}
        last_stage_ns[label] = res.exec_time_ns
    return res


def kernel(**inputs):
    global last_hw_ns, last_stage_ns
    last_stage_ns = None
    f32 = np.float32
    x = np.asarray(inputs["x"], f32)
    conv_w = np.asarray(inputs["conv_w"], f32)
    conv_b = np.asarray(inputs["conv_b"], f32)
    w1 = np.asarray(inputs["w1"], f32)
    b1 = np.asarray(inputs["b1"], f32)
    g1, be1 = np.asarray(inputs["g1"], f32), np.asarray(inputs["be1"], f32)
    m1, v1 = np.asarray(inputs["m1"], f32), np.asarray(inputs["v1"], f32)
    w2 = np.asarray(inputs["w2"], f32)
    b2 = np.asarray(inputs["b2"], f32)
    g2, be2 = np.asarray(inputs["g2"], f32), np.asarray(inputs["be2"], f32)
    m2, v2 = np.asarray(inputs["m2"], f32), np.asarray(inputs["v2"], f32)
    w3 = np.asarray(inputs["w3"], f32)
    b3 = np.asarray(inputs["b3"], f32)

    # ---- stage 1: ConvLSTM (batch-parallel)
    nc1 = _build_stage1()
    maps1 = _prep_stage1_inputs(x, conv_w, conv_b)
    res1 = _run(nc1, maps1, "stage1")
    import ml_dtypes
    bf = ml_dtypes.bfloat16
    hs_all = np.stack([res1.results[c]["hs"] for c in range(NC)])  # (8,S,H,32,L) bf16

    # ---- reshard: (8,S,H,32,L) -> flatT (S*H*L, 256), feature-major, bf16
    flatT = np.ascontiguousarray(
        hs_all.transpose(1, 2, 4, 0, 3)
    ).reshape(KTOT, B)

    # ---- stage 2: big GEMM, contract-dim sharded
    s1 = g1 / np.sqrt(v1 + EPS)
    c1 = b1 * s1 + (be1 - m1 * s1)
    w1sT = np.ascontiguousarray((w1 * s1[:, None]).T).astype(bf)    # (KTOT, 768)
    nc2 = _build_stage2()
    maps2 = []
    for c in range(NC):
        sl = slice(c * KSH, (c + 1) * KSH)
        maps2.append({
            "w1p": np.ascontiguousarray(w1sT[sl]).reshape(KCH, 128, 768),
            "ft": np.ascontiguousarray(flatT[sl]).reshape(KCH, 128, B),
        })
    res2 = _run(nc2, maps2, "stage2")
    y1 = np.sum([res2.results[c]["y1p"] for c in range(NC)], axis=0,
                dtype=np.float64).astype(f32)                       # (768, 256)

    # ---- stage 3: epilogue (batch-parallel)
    s2 = g2 / np.sqrt(v2 + EPS)
    c2 = b2 * s2 + (be2 - m2 * s2)
    c1t = np.ascontiguousarray(c1.reshape(6, 128).T, f32)           # (128, 6)
    w2p = np.ascontiguousarray(
        (w2 * s2[:, None]).T.reshape(6, 128, 12), f32
    )
    w3e = np.concatenate([w3.T, b3[None, :]], axis=0).astype(f32)   # (13, 10)
    nc3 = _build_stage3()
    maps3 = []
    for c in range(NC):
        ysl = np.ascontiguousarray(
            y1[:, c * BLOC : (c + 1) * BLOC]
        ).reshape(6, 128, BLOC)
        maps3.append({
            "y1s": ysl, "c1t": c1t, "w2p": w2p,
            "c2t": c2.reshape(12, 1).astype(f32), "w3e": w3e,
        })
    res3 = _run(nc3, maps3, "stage3")
    y3 = np.concatenate([res3.results[c]["y3p"] for c in range(NC)], axis=0)
    if last_stage_ns and all(v is not None for v in last_stage_ns.values()):
        last_hw_ns = sum(last_stage_ns.values())
    return np.ascontiguousarray(y3, f32)


# revision 8
# speedup vs baseline: 38016.7877x; 1.1958x over previous
"""Trainium2 Bass kernel for nn_CropConvLSTM.

Model: ConvLSTM (Conv1d(1+H -> 4H, k=3, pad=1), S=12 steps) over x (B=256,
S=12, L=128), then head Linear(98304->768)+BN+ReLU, Linear(768->12)+BN+ReLU,
Linear(12->10).

Distribution over 8 NeuronCores, three launches:
  Stage 1: ConvLSTM, data-parallel over batch (32 samples/core). Conv done as
    3 shifted fp32r matmuls (K=66: 64 h-channels + x-row + ones-row for the
    conv bias) accumulating in PSUM; gates on ACT/DVE/GPSIMD with all tensors
    at legal partition bases (tanh computed as 2*sigmoid(2x)-1 so a single
    per-partition-scaled sigmoid covers the [o;g] psum tile).
  Stage 2: y1 = flat @ (w1*bn1_scale).T, sharded over the 98304 contract dim
    (12288 features/core); each core emits a partial (768, 256), host reduces.
  Stage 3: bias+ReLU, Linear2+BN+ReLU, Linear3 (+b3 via ones-row trick),
    data-parallel over batch again.

BN (eval mode) is folded into the weights/biases on the host.
"""
import os
import sys

sys.path.insert(0, "/opt/trn_rl_repo")

from functools import lru_cache

import numpy as np

import concourse.bass as bass
import concourse.tile as tile
from concourse import bacc, mybir
from concourse.bass_utils import run_bass_kernel_spmd

F32 = mybir.dt.float32
F32R = mybir.dt.float32r
BF16 = mybir.dt.bfloat16
AF = mybir.ActivationFunctionType

B, S, L, H, C = 256, 12, 128, 64, 10
NC = 8
BLOC = B // NC            # 32 samples per core in stages 1/3
KTOT = S * H * L          # 98304
KSH = KTOT // NC          # 12288 contract features per core in stage 2
KCH = KSH // 128          # 96 k-chunks per core
EPS = 1e-5
CORE_IDS = list(range(NC))


# ---------------------------------------------------------------- stage 1
@lru_cache(maxsize=1)
def _build_stage1():
    nc = bacc.Bacc("TRN2", target_bir_lowering=False, debug=False, num_devices=NC)
    xp = nc.dram_tensor("xp", [S, BLOC, L], BF16, kind="ExternalInput").ap()
    wfi = nc.dram_tensor("wfi", [66, 3, 128], BF16, kind="ExternalInput").ap()
    wif = nc.dram_tensor("wif", [66, 3, 128], BF16, kind="ExternalInput").ap()
    wog = nc.dram_tensor("wog", [66, 3, 128], BF16, kind="ExternalInput").ap()
    hs = nc.dram_tensor("hs", [S, H, BLOC, L], BF16, kind="ExternalOutput").ap()

    HB = BLOC // 2        # 16 samples per half-step

    with tile.TileContext(nc) as tc:
        with (
            tc.tile_pool(name="persist", bufs=1) as pp,
            tc.tile_pool(name="sig", bufs=4) as sp,
            tc.tile_pool(name="tmp", bufs=2) as tp,
            tc.tile_pool(name="psfi", bufs=2, space="PSUM") as ps_fi,
            tc.tile_pool(name="psog", bufs=2, space="PSUM") as ps_og,
        ):
            comb = pp.tile([66, BLOC, L + 2], BF16)     # [h(64); x; ones]
            cpk = pp.tile([128, HB, L], BF16)           # c: [h1 | h2] packed
            wt_fi = pp.tile([66, 3, 128], BF16)
            wt_if = pp.tile([66, 3, 128], BF16)
            wt_og = pp.tile([66, 3, 128], BF16)
            scv = pp.tile([128, 1], F32)                # act scale [1;2]

            nc.vector.memset(comb, 0.0)
            # ones row is partition 65; engine bases must be 32-aligned, so
            # set [64:66] to 1.0 then zero the x row (64) again
            nc.vector.memset(comb[64:66], 1.0)
            nc.vector.memset(comb[64:65], 0.0)
            nc.vector.memset(cpk, 0.0)
            nc.vector.memset(scv[0:64], 1.0)
            nc.vector.memset(scv[64:128], 2.0)
            nc.sync.dma_start(out=wt_fi, in_=wfi)
            nc.sync.dma_start(out=wt_if, in_=wif)
            nc.sync.dma_start(out=wt_og, in_=wog)

            for s in range(S):
                # x for this step -> partition 64, data cols [1, 129)
                nc.sync.dma_start(out=comb[64:65, :, 1 : L + 1], in_=xp[s : s + 1])

                sgate = []   # per-half [f/i or i/f] sigmoid tiles
                sog = []     # per-half [sig(o); sig(2g)] tiles
                tg = tp.tile([128, HB, L], BF16)    # tanh(g): [h2 | h1]
                t1 = tp.tile([128, HB, L], BF16)    # sf*c:   [h1 | h2]
                t2 = tp.tile([128, HB, L], BF16)    # si*tg:  [h1 | h2]
                for half in range(2):
                    s0 = half * HB
                    wgate = wt_fi if half == 0 else wt_if
                    sg = sp.tile([128, HB, L], BF16, name="sgate")
                    # keep [sig(o); sig(2g)] in fp32: the 2x-1 unfold would
                    # amplify bf16 rounding of sig into absolute tanh error
                    so = sp.tile([128, HB, L], F32, name="sog")
                    # quarter-granular PSUM (2 banks/tile) so the PE streams
                    # ahead while ACT drains the previous quarter
                    for q in range(2):
                        q0 = q * 8
                        pfi = ps_fi.tile([128, 8, L], F32, name="pfi")
                        pog = ps_og.tile([128, 8, L], F32, name="pog")
                        for chunk in range(2):
                            c0 = chunk * 4
                            for t in range(3):
                                rhs = comb[:, s0 + q0 + c0 : s0 + q0 + c0 + 4,
                                           t : t + L]
                                st, sp_ = (t == 0), (t == 2)
                                nc.tensor.matmul(
                                    pfi[:, c0 : c0 + 4, :], lhsT=wgate[:, t, :],
                                    rhs=rhs, start=st, stop=sp_,
                                )
                                nc.tensor.matmul(
                                    pog[:, c0 : c0 + 4, :], lhsT=wt_og[:, t, :],
                                    rhs=rhs, start=st, stop=sp_,
                                )
                        nc.scalar.activation(sg[:, q0 : q0 + 8, :], pfi, AF.Sigmoid)
                        nc.scalar.activation(so[:, q0 : q0 + 8, :], pog,
                                             AF.Sigmoid, scale=scv)
                    sgate.append(sg)
                    sog.append(so)
                    # tanh(g) = 2*sig(2g) - 1 ; h1 -> tg[64:], h2 -> tg[:64]
                    dst = tg[64:128] if half == 0 else tg[0:64]
                    nc.vector.tensor_scalar(
                        out=dst, in0=so[64:128], scalar1=2.0, scalar2=-1.0,
                        op0=mybir.AluOpType.mult, op1=mybir.AluOpType.add,
                    )
                # h1: sf = sgate[0][0:64], c_h1 = cpk[0:64]   -> t1[0:64]
                # h2: sf = sgate[1][64:128], c_h2 = cpk[64:128] -> t1[64:128]
                nc.vector.tensor_mul(t1[0:64], sgate[0][0:64], cpk[0:64])
                nc.vector.tensor_mul(t1[64:128], sgate[1][64:128], cpk[64:128])
                # h1: si = sgate[0][64:128], tg_h1 = tg[64:128] -> t2[0:64]
                # h2: si = sgate[1][0:64],   tg_h2 = tg[0:64]   -> t2[64:128]
                nc.vector.tensor_mul(t2[0:64], sgate[0][64:128], tg[64:128])
                nc.vector.tensor_mul(t2[64:128], sgate[1][0:64], tg[0:64])
                nc.vector.tensor_add(cpk, t1, t2)
                # tanh(c) = 2*sig(2c) - 1 (fp32 intermediate, see above)
                tcs = tp.tile([128, HB, L], F32, name="tcs")
                nc.scalar.activation(tcs, cpk, AF.Sigmoid, scale=2.0)
                tcf = tp.tile([64, BLOC, L], BF16, name="tcf")
                nc.vector.tensor_scalar(
                    out=tcf[:, 0:HB, :], in0=tcs[0:64], scalar1=2.0, scalar2=-1.0,
                    op0=mybir.AluOpType.mult, op1=mybir.AluOpType.add,
                )
                nc.vector.tensor_scalar(
                    out=tcf[:, HB:BLOC, :], in0=tcs[64:128], scalar1=2.0,
                    scalar2=-1.0, op0=mybir.AluOpType.mult, op1=mybir.AluOpType.add,
                )
                # h = sig(o) * tanh(c) -> comb h rows (next step input)
                for half in range(2):
                    s0 = half * HB
                    nc.vector.tensor_mul(
                        comb[0:64, s0 : s0 + HB, 1 : L + 1],
                        sog[half][0:64],
                        tcf[:, s0 : s0 + HB, :],
                    )
                    nc.sync.dma_start(
                        out=hs[s, :, s0 : s0 + HB, :],
                        in_=comb[0:64, s0 : s0 + HB, 1 : L + 1],
                    )
    nc.compile()
    return nc


# ---------------------------------------------------------------- stage 2
@lru_cache(maxsize=1)
def _build_stage2():
    nc = bacc.Bacc("TRN2", target_bir_lowering=False, debug=False, num_devices=NC)
    w1p = nc.dram_tensor("w1p", [KCH, 128, 768], BF16, kind="ExternalInput").ap()
    ft = nc.dram_tensor("ft", [KCH, 128, B], BF16, kind="ExternalInput").ap()
    y1p = nc.dram_tensor("y1p", [768, B], F32, kind="ExternalOutput").ap()

    KB = 4                       # k-chunks per DMA batch
    NB = KCH // KB               # 24 batches

    with tile.TileContext(nc) as tc:
        with (
            tc.tile_pool(name="wp", bufs=3) as wp,
            tc.tile_pool(name="rp", bufs=3) as rp,
            tc.tile_pool(name="op", bufs=2) as op,
            tc.tile_pool(name="ps", bufs=1, space="PSUM") as ps,
        ):
            acc = [ps.tile([128, B], F32, name=f"acc{m}") for m in range(6)]
            for kb in range(NB):
                wt = wp.tile([128, KB, 768], BF16, name="wt")
                rt = rp.tile([128, KB, B], BF16, name="rt")
                nc.sync.dma_start(
                    out=wt,
                    in_=w1p[kb * KB : (kb + 1) * KB].rearrange("k p m -> p k m"),
                )
                nc.gpsimd.dma_start(
                    out=rt,
                    in_=ft[kb * KB : (kb + 1) * KB].rearrange("k p b -> p k b"),
                )
                for kc in range(KB):
                    for m in range(6):
                        nc.tensor.matmul(
                            acc[m], lhsT=wt[:, kc, m * 128 : (m + 1) * 128],
                            rhs=rt[:, kc, :],
                            start=(kb == 0 and kc == 0),
                            stop=(kb == NB - 1 and kc == KB - 1),
                        )
            for m in range(6):
                ot = op.tile([128, B], F32, name="ot")
                nc.vector.tensor_copy(ot, acc[m])
                nc.sync.dma_start(out=y1p[m * 128 : (m + 1) * 128], in_=ot)
    nc.compile()
    return nc


# ---------------------------------------------------------------- stage 3
@lru_cache(maxsize=1)
def _build_stage3():
    nc = bacc.Bacc("TRN2", target_bir_lowering=False, debug=False, num_devices=NC)
    y1s = nc.dram_tensor("y1s", [6, 128, BLOC], F32R, kind="ExternalInput").ap()
    c1t = nc.dram_tensor("c1t", [128, 6], F32, kind="ExternalInput").ap()
    w2p = nc.dram_tensor("w2p", [6, 128, 12], F32R, kind="ExternalInput").ap()
    c2t = nc.dram_tensor("c2t", [12, 1], F32, kind="ExternalInput").ap()
    w3e = nc.dram_tensor("w3e", [13, 10], F32R, kind="ExternalInput").ap()
    y3p = nc.dram_tensor("y3p", [BLOC, C], F32, kind="ExternalOutput").ap()

    with tile.TileContext(nc) as tc:
        with (
            tc.tile_pool(name="sb", bufs=1) as sb,
            tc.tile_pool(name="ps", bufs=1, space="PSUM") as ps,
        ):
            yt = sb.tile([128, 6, BLOC], F32R)
            c1 = sb.tile([128, 6], F32)
            w2t = sb.tile([128, 6, 12], F32R)
            c2 = sb.tile([12, 1], F32)
            w3t = sb.tile([13, 10], F32R)
            nc.sync.dma_start(out=yt, in_=y1s.rearrange("k p b -> p k b"))
            nc.sync.dma_start(out=c1, in_=c1t)
            nc.sync.dma_start(out=w2t, in_=w2p.rearrange("k p m -> p k m"))
            nc.sync.dma_start(out=c2, in_=c2t)
            nc.sync.dma_start(out=w3t, in_=w3e)

            r1 = sb.tile([128, 6, BLOC], F32R)
            for kc in range(6):
                nc.scalar.activation(
                    r1[:, kc, :], yt[:, kc, :], AF.Relu, bias=c1[:, kc : kc + 1]
                )
            p2 = ps.tile([12, BLOC], F32)
            for kc in range(6):
                nc.tensor.matmul(
                    p2, lhsT=w2t[:, kc, :], rhs=r1[:, kc, :],
                    start=(kc == 0), stop=(kc == 5),
                )
            r2 = sb.tile([13, BLOC], F32R)
            # ones row lives at partition 12 (not 32-aligned): fill the whole
            # tile with 1.0 first, then overwrite rows 0..11 via ACT
            nc.vector.memset(r2.bitcast(F32), 1.0)
            nc.scalar.activation(r2[0:12], p2, AF.Relu, bias=c2)
            p3 = ps.tile([BLOC, C], F32)
            nc.tensor.matmul(p3, lhsT=r2, rhs=w3t, start=True, stop=True)
            ot = sb.tile([BLOC, C], F32)
            nc.vector.tensor_copy(ot, p3)
            nc.sync.dma_start(out=y3p, in_=ot)
    nc.compile()
    return nc


# ---------------------------------------------------------------- host glue
def _prep_stage1_inputs(x, conv_w, conv_b):
    """Build per-core stage-1 in_maps. conv_w: (4H, 1+H, 3); ci order in the
    reference combined tensor is [x, h0..h63]; our comb rows are
    [h0..h63, x, ones]."""
    import ml_dtypes
    bf = ml_dtypes.bfloat16
    f32 = np.float32
    ci_perm = np.concatenate([np.arange(1, 65), [0]])       # comb rows 0..64
    w = np.ascontiguousarray(conv_w[:, ci_perm, :], f32)    # (256, 65, 3)
    bi, bf_, bo, bg = (conv_b[0:64], conv_b[64:128],
                       conv_b[128:192], conv_b[192:256])
    wi, wf, wo, wg = w[0:64], w[64:128], w[128:192], w[192:256]

    def lhst(wa, wb, ba, bb):
        # -> (66, 3, 128): rows = [65 ci rows, bias row]; cols = [A(64)|B(64)]
        out = np.zeros((66, 3, 128), f32)
        out[0:65, :, 0:64] = wa.transpose(1, 2, 0)
        out[0:65, :, 64:128] = wb.transpose(1, 2, 0)
        out[65, 1, 0:64] = ba
        out[65, 1, 64:128] = bb
        return out

    wfi = lhst(wf, wi, bf_, bi).astype(bf)
    wif = lhst(wi, wf, bi, bf_).astype(bf)
    wog = lhst(wo, wg, bo, bg).astype(bf)
    maps = []
    for c in range(NC):
        xc = np.ascontiguousarray(
            x[c * BLOC : (c + 1) * BLOC].transpose(1, 0, 2)
        ).astype(bf)  # (S, BLOC, L)
        maps.append({"xp": xc, "wfi": wfi, "wif": wif, "wog": wog})
    return maps


last_hw_ns = None
last_stage_ns = None


def _run(nc, maps, label):
    trace = bool(int(os.environ.get("BASSK_TRACE", "0")))
    res = run_bass_kernel_spmd(nc, maps, core_ids=CORE_IDS, trace=trace)
    if trace:
        global last_stage_ns
        if last_stage_ns is None:
            last_stage_ns = {}
        last_stage_ns[label] = res.exec_time_ns
    return res


def kernel(**inputs):
    global last_hw_ns, last_stage_ns
    last_stage_ns = None
    f32 = np.float32
    x = np.asarray(inputs["x"], f32)
    conv_w = np.asarray(inputs["conv_w"], f32)
    conv_b = np.asarray(inputs["conv_b"], f32)
    w1 = np.asarray(inputs["w1"], f32)
    b1 = np.asarray(inputs["b1"], f32)
    g1, be1 = np.asarray(inputs["g1"], f32), np.asarray(inputs["be1"], f32)
    m1, v1 = np.asarray(inputs["m1"], f32), np.asarray(inputs["v1"], f32)
    w2 = np.asarray(inputs["w2"], f32)
    b2 = np.asarray(inputs["b2"], f32)
    g2, be2 = np.asarray(inputs["g2"], f32), np.asarray(inputs["be2"], f32)
    m2, v2 = np.asarray(inputs["m2"], f32), np.asarray(inputs["v2"], f32)
    w3 = np.asarray(inputs["w3"], f32)
    b3 = np.asarray(inputs["b3"], f32)

    # ---- stage 1: ConvLSTM (batch-parallel)
    nc1 = _build_stage1()
    maps1 = _prep_stage1_inputs(x, conv_w, conv_b)
    res1 = _run(nc1, maps1, "stage1")
    import ml_dtypes
    bf = ml_dtypes.bfloat16
    hs_all = np.stack([res1.results[c]["hs"] for c in range(NC)])  # (8,S,H,32,L) bf16

    # ---- reshard: (8,S,H,32,L) -> flatT (S*H*L, 256), feature-major, bf16
    flatT = np.ascontiguousarray(
        hs_all.transpose(1, 2, 4, 0, 3)
    ).reshape(KTOT, B)

    # ---- stage 2: big GEMM, contract-dim sharded
    s1 = g1 / np.sqrt(v1 + EPS)
    c1 = b1 * s1 + (be1 - m1 * s1)
    w1sT = np.ascontiguousarray((w1 * s1[:, None]).T).astype(bf)    # (KTOT, 768)
    nc2 = _build_stage2()
    maps2 = []
    for c in range(NC):
        sl = slice(c * KSH, (c + 1) * KSH)
        maps2.append({
            "w1p": np.ascontiguousarray(w1sT[sl]).reshape(KCH, 128, 768),
            "ft": np.ascontiguousarray(flatT[sl]).reshape(KCH, 128, B),
        })
    res2 = _run(nc2, maps2, "stage2")
    y1 = np.sum([res2.results[c]["y1p"] for c in range(NC)], axis=0,
                dtype=np.float64).astype(f32)                       # (768, 256)

    # ---- stage 3: epilogue (batch-parallel)
    s2 = g2 / np.sqrt(v2 + EPS)
    c2 = b2 * s2 + (be2 - m2 * s2)
    c1t = np.ascontiguousarray(c1.reshape(6, 128).T, f32)           # (128, 6)
    w2p = np.ascontiguousarray(
        (w2 * s2[:, None]).T.reshape(6, 128, 12), f32
    )
    w3e = np.concatenate([w3.T, b3[None, :]], axis=0).astype(f32)   # (13, 10)
    nc3 = _build_stage3()
    maps3 = []
    for c in range(NC):
        ysl = np.ascontiguousarray(
            y1[:, c * BLOC : (c + 1) * BLOC]
        ).reshape(6, 128, BLOC)
        maps3.append({
            "y1s": ysl, "c1t": c1t, "w2p": w2p,
            "c2t": c2.reshape(12, 1).astype(f32), "w3e": w3e,
        })
    res3 = _run(nc3, maps3, "stage3")
    y3 = np.concatenate([res3.results[c]["y3p"] for c in range(NC)], axis=0)
    if last_stage_ns and all(v is not None for v in last_stage_ns.values()):
        last_hw_ns = sum(last_stage_ns.values())
    return np.ascontiguousarray(y3, f32)


# revision 9
# speedup vs baseline: 41648.4464x; 1.0955x over previous
"""Trainium2 Bass kernel for nn_CropConvLSTM.

Model: ConvLSTM (Conv1d(1+H -> 4H, k=3, pad=1), S=12 steps) over x (B=256,
S=12, L=128), then head Linear(98304->768)+BN+ReLU, Linear(768->12)+BN+ReLU,
Linear(12->10).

Distribution over 8 NeuronCores, three launches:
  Stage 1: ConvLSTM, data-parallel over batch (32 samples/core). Conv done as
    3 shifted fp32r matmuls (K=66: 64 h-channels + x-row + ones-row for the
    conv bias) accumulating in PSUM; gates on ACT/DVE/GPSIMD with all tensors
    at legal partition bases (tanh computed as 2*sigmoid(2x)-1 so a single
    per-partition-scaled sigmoid covers the [o;g] psum tile).
  Stage 2: y1 = flat @ (w1*bn1_scale).T, sharded over the 98304 contract dim
    (12288 features/core); each core emits a partial (768, 256), host reduces.
  Stage 3: bias+ReLU, Linear2+BN+ReLU, Linear3 (+b3 via ones-row trick),
    data-parallel over batch again.

BN (eval mode) is folded into the weights/biases on the host.
"""
import os
import sys

sys.path.insert(0, "/opt/trn_rl_repo")

from functools import lru_cache

import numpy as np

import concourse.bass as bass
import concourse.tile as tile
from concourse import bacc, mybir
from concourse.bass_utils import run_bass_kernel_spmd

F32 = mybir.dt.float32
F32R = mybir.dt.float32r
BF16 = mybir.dt.bfloat16
AF = mybir.ActivationFunctionType

B, S, L, H, C = 256, 12, 128, 64, 10
NC = 8
BLOC = B // NC            # 32 samples per core in stages 1/3
KTOT = S * H * L          # 98304
KSH = KTOT // NC          # 12288 contract features per core in stage 2
KCH = KSH // 128          # 96 k-chunks per core
EPS = 1e-5
CORE_IDS = list(range(NC))


# ---------------------------------------------------------------- stage 1
@lru_cache(maxsize=1)
def _build_stage1():
    nc = bacc.Bacc("TRN2", target_bir_lowering=False, debug=False, num_devices=NC)
    xp = nc.dram_tensor("xp", [S, BLOC, L], BF16, kind="ExternalInput").ap()
    wfi = nc.dram_tensor("wfi", [66, 3, 128], BF16, kind="ExternalInput").ap()
    wif = nc.dram_tensor("wif", [66, 3, 128], BF16, kind="ExternalInput").ap()
    wog = nc.dram_tensor("wog", [66, 3, 128], BF16, kind="ExternalInput").ap()
    hs = nc.dram_tensor("hs", [S, H, BLOC, L], BF16, kind="ExternalOutput").ap()

    HB = BLOC // 2        # 16 samples per half-step

    with tile.TileContext(nc) as tc:
        with (
            tc.tile_pool(name="persist", bufs=1) as pp,
            tc.tile_pool(name="sig", bufs=4) as sp,
            tc.tile_pool(name="tmp", bufs=2) as tp,
            tc.tile_pool(name="psfi", bufs=2, space="PSUM") as ps_fi,
            tc.tile_pool(name="psog", bufs=2, space="PSUM") as ps_og,
        ):
            comb = pp.tile([66, BLOC, L + 2], BF16)     # [h(64); x; ones]
            cpk = pp.tile([128, HB, L], BF16)           # c: [h1 | h2] packed
            wt_fi = pp.tile([66, 3, 128], BF16)
            wt_if = pp.tile([66, 3, 128], BF16)
            wt_og = pp.tile([66, 3, 128], BF16)
            scv = pp.tile([128, 1], F32)                # act scale [1;2]

            nc.vector.memset(comb, 0.0)
            # ones row is partition 65; engine bases must be 32-aligned, so
            # set [64:66] to 1.0 then zero the x row (64) again
            nc.vector.memset(comb[64:66], 1.0)
            nc.vector.memset(comb[64:65], 0.0)
            nc.vector.memset(cpk, 0.0)
            nc.vector.memset(scv[0:64], 1.0)
            nc.vector.memset(scv[64:128], 2.0)
            nc.sync.dma_start(out=wt_fi, in_=wfi)
            nc.sync.dma_start(out=wt_if, in_=wif)
            nc.sync.dma_start(out=wt_og, in_=wog)

            for s in range(S):
                # x for this step -> partition 64, data cols [1, 129)
                nc.sync.dma_start(out=comb[64:65, :, 1 : L + 1], in_=xp[s : s + 1])

                sgate = []   # per-half [f/i or i/f] sigmoid tiles
                sog = []     # per-half [sig(o); sig(2g)] tiles
                tg = tp.tile([128, HB, L], BF16)    # tanh(g): [h2 | h1]
                t1 = tp.tile([128, HB, L], BF16)    # sf*c:   [h1 | h2]
                t2 = tp.tile([128, HB, L], BF16)    # si*tg:  [h1 | h2]
                for half in range(2):
                    s0 = half * HB
                    wgate = wt_fi if half == 0 else wt_if
                    sg = sp.tile([128, HB, L], BF16, name="sgate")
                    # keep [sig(o); sig(2g)] in fp32: the 2x-1 unfold would
                    # amplify bf16 rounding of sig into absolute tanh error
                    so = sp.tile([128, HB, L], F32, name="sog")
                    # quarter-granular PSUM (2 banks/tile) so the PE streams
                    # ahead while ACT drains the previous quarter
                    for q in range(2):
                        q0 = q * 8
                        pfi = ps_fi.tile([128, 8, L], F32, name="pfi")
                        pog = ps_og.tile([128, 8, L], F32, name="pog")
                        for chunk in range(2):
                            c0 = chunk * 4
                            for t in range(3):
                                rhs = comb[:, s0 + q0 + c0 : s0 + q0 + c0 + 4,
                                           t : t + L]
                                st, sp_ = (t == 0), (t == 2)
                                nc.tensor.matmul(
                                    pfi[:, c0 : c0 + 4, :], lhsT=wgate[:, t, :],
                                    rhs=rhs, start=st, stop=sp_,
                                )
                                nc.tensor.matmul(
                                    pog[:, c0 : c0 + 4, :], lhsT=wt_og[:, t, :],
                                    rhs=rhs, start=st, stop=sp_,
                                )
                        nc.scalar.activation(sg[:, q0 : q0 + 8, :], pfi, AF.Sigmoid)
                        nc.scalar.activation(so[:, q0 : q0 + 8, :], pog,
                                             AF.Sigmoid, scale=scv)
                    sgate.append(sg)
                    sog.append(so)
                    # tanh(g) = 2*sig(2g) - 1 ; h1 -> tg[64:], h2 -> tg[:64]
                    dst = tg[64:128] if half == 0 else tg[0:64]
                    nc.vector.tensor_scalar(
                        out=dst, in0=so[64:128], scalar1=2.0, scalar2=-1.0,
                        op0=mybir.AluOpType.mult, op1=mybir.AluOpType.add,
                    )
                    # per-half gate chain so this half's h lands while the
                    # other half's matmuls keep the PE busy
                    lo, hi = (slice(0, 64), slice(64, 128))
                    m, o_ = (lo, hi) if half == 0 else (hi, lo)
                    # t1 = sig(f)*c ; t2 = sig(i)*tanh(g)  (write at half block)
                    nc.vector.tensor_mul(t1[m], sg[m], cpk[m])
                    nc.vector.tensor_mul(t2[m], sg[o_], tg[o_])
                    nc.vector.tensor_add(cpk[m], t1[m], t2[m])
                    # tanh(c) = 2*sig(2c) - 1 (fp32 intermediate, see above)
                    tcs = tp.tile([64, HB, L], F32, name="tcs")
                    nc.scalar.activation(tcs, cpk[m], AF.Sigmoid, scale=2.0)
                    tcf = tp.tile([64, HB, L], BF16, name="tcf")
                    nc.vector.tensor_scalar(
                        out=tcf, in0=tcs, scalar1=2.0, scalar2=-1.0,
                        op0=mybir.AluOpType.mult, op1=mybir.AluOpType.add,
                    )
                    # h = sig(o) * tanh(c) -> comb h rows (next step input)
                    nc.vector.tensor_mul(
                        comb[0:64, s0 : s0 + HB, 1 : L + 1], so[0:64], tcf,
                    )
                    nc.sync.dma_start(
                        out=hs[s, :, s0 : s0 + HB, :],
                        in_=comb[0:64, s0 : s0 + HB, 1 : L + 1],
                    )
    nc.compile()
    return nc


# ---------------------------------------------------------------- stage 2
@lru_cache(maxsize=1)
def _build_stage2():
    nc = bacc.Bacc("TRN2", target_bir_lowering=False, debug=False, num_devices=NC)
    w1p = nc.dram_tensor("w1p", [KCH, 128, 768], BF16, kind="ExternalInput").ap()
    ft = nc.dram_tensor("ft", [KCH, 128, B], BF16, kind="ExternalInput").ap()
    y1p = nc.dram_tensor("y1p", [768, B], F32, kind="ExternalOutput").ap()

    KB = 4                       # k-chunks per DMA batch
    NB = KCH // KB               # 24 batches

    with tile.TileContext(nc) as tc:
        with (
            tc.tile_pool(name="wp", bufs=3) as wp,
            tc.tile_pool(name="rp", bufs=3) as rp,
            tc.tile_pool(name="op", bufs=2) as op,
            tc.tile_pool(name="ps", bufs=1, space="PSUM") as ps,
        ):
            acc = [ps.tile([128, B], F32, name=f"acc{m}") for m in range(6)]
            for kb in range(NB):
                wt = wp.tile([128, KB, 768], BF16, name="wt")
                rt = rp.tile([128, KB, B], BF16, name="rt")
                nc.sync.dma_start(
                    out=wt,
                    in_=w1p[kb * KB : (kb + 1) * KB].rearrange("k p m -> p k m"),
                )
                nc.gpsimd.dma_start(
                    out=rt,
                    in_=ft[kb * KB : (kb + 1) * KB].rearrange("k p b -> p k b"),
                )
                for kc in range(KB):
                    for m in range(6):
                        nc.tensor.matmul(
                            acc[m], lhsT=wt[:, kc, m * 128 : (m + 1) * 128],
                            rhs=rt[:, kc, :],
                            start=(kb == 0 and kc == 0),
                            stop=(kb == NB - 1 and kc == KB - 1),
                        )
            for m in range(6):
                ot = op.tile([128, B], F32, name="ot")
                nc.vector.tensor_copy(ot, acc[m])
                nc.sync.dma_start(out=y1p[m * 128 : (m + 1) * 128], in_=ot)
    nc.compile()
    return nc


# ---------------------------------------------------------------- stage 3
@lru_cache(maxsize=1)
def _build_stage3():
    nc = bacc.Bacc("TRN2", target_bir_lowering=False, debug=False, num_devices=NC)
    y1s = nc.dram_tensor("y1s", [6, 128, BLOC], F32R, kind="ExternalInput").ap()
    c1t = nc.dram_tensor("c1t", [128, 6], F32, kind="ExternalInput").ap()
    w2p = nc.dram_tensor("w2p", [6, 128, 12], F32R, kind="ExternalInput").ap()
    c2t = nc.dram_tensor("c2t", [12, 1], F32, kind="ExternalInput").ap()
    w3e = nc.dram_tensor("w3e", [13, 10], F32R, kind="ExternalInput").ap()
    y3p = nc.dram_tensor("y3p", [BLOC, C], F32, kind="ExternalOutput").ap()

    with tile.TileContext(nc) as tc:
        with (
            tc.tile_pool(name="sb", bufs=1) as sb,
            tc.tile_pool(name="ps", bufs=1, space="PSUM") as ps,
        ):
            yt = sb.tile([128, 6, BLOC], F32R)
            c1 = sb.tile([128, 6], F32)
            w2t = sb.tile([128, 6, 12], F32R)
            c2 = sb.tile([12, 1], F32)
            w3t = sb.tile([13, 10], F32R)
            nc.sync.dma_start(out=yt, in_=y1s.rearrange("k p b -> p k b"))
            nc.sync.dma_start(out=c1, in_=c1t)
            nc.sync.dma_start(out=w2t, in_=w2p.rearrange("k p m -> p k m"))
            nc.sync.dma_start(out=c2, in_=c2t)
            nc.sync.dma_start(out=w3t, in_=w3e)

            r1 = sb.tile([128, 6, BLOC], F32R)
            for kc in range(6):
                nc.scalar.activation(
                    r1[:, kc, :], yt[:, kc, :], AF.Relu, bias=c1[:, kc : kc + 1]
                )
            p2 = ps.tile([12, BLOC], F32)
            for kc in range(6):
                nc.tensor.matmul(
                    p2, lhsT=w2t[:, kc, :], rhs=r1[:, kc, :],
                    start=(kc == 0), stop=(kc == 5),
                )
            r2 = sb.tile([13, BLOC], F32R)
            # ones row lives at partition 12 (not 32-aligned): fill the whole
            # tile with 1.0 first, then overwrite rows 0..11 via ACT
            nc.vector.memset(r2.bitcast(F32), 1.0)
            nc.scalar.activation(r2[0:12], p2, AF.Relu, bias=c2)
            p3 = ps.tile([BLOC, C], F32)
            nc.tensor.matmul(p3, lhsT=r2, rhs=w3t, start=True, stop=True)
            ot = sb.tile([BLOC, C], F32)
            nc.vector.tensor_copy(ot, p3)
            nc.sync.dma_start(out=y3p, in_=ot)
    nc.compile()
    return nc


# ---------------------------------------------------------------- host glue
def _prep_stage1_inputs(x, conv_w, conv_b):
    """Build per-core stage-1 in_maps. conv_w: (4H, 1+H, 3); ci order in the
    reference combined tensor is [x, h0..h63]; our comb rows are
    [h0..h63, x, ones]."""
    import ml_dtypes
    bf = ml_dtypes.bfloat16
    f32 = np.float32
    ci_perm = np.concatenate([np.arange(1, 65), [0]])       # comb rows 0..64
    w = np.ascontiguousarray(conv_w[:, ci_perm, :], f32)    # (256, 65, 3)
    bi, bf_, bo, bg = (conv_b[0:64], conv_b[64:128],
                       conv_b[128:192], conv_b[192:256])
    wi, wf, wo, wg = w[0:64], w[64:128], w[128:192], w[192:256]

    def lhst(wa, wb, ba, bb):
        # -> (66, 3, 128): rows = [65 ci rows, bias row]; cols = [A(64)|B(64)]
        out = np.zeros((66, 3, 128), f32)
        out[0:65, :, 0:64] = wa.transpose(1, 2, 0)
        out[0:65, :, 64:128] = wb.transpose(1, 2, 0)
        out[65, 1, 0:64] = ba
        out[65, 1, 64:128] = bb
        return out

    wfi = lhst(wf, wi, bf_, bi).astype(bf)
    wif = lhst(wi, wf, bi, bf_).astype(bf)
    wog = lhst(wo, wg, bo, bg).astype(bf)
    maps = []
    for c in range(NC):
        xc = np.ascontiguousarray(
            x[c * BLOC : (c + 1) * BLOC].transpose(1, 0, 2)
        ).astype(bf)  # (S, BLOC, L)
        maps.append({"xp": xc, "wfi": wfi, "wif": wif, "wog": wog})
    return maps


last_hw_ns = None
last_stage_ns = None


def _run(nc, maps, label):
    trace = bool(int(os.environ.get("BASSK_TRACE", "0")))
    res = run_bass_kernel_spmd(nc, maps, core_ids=CORE_IDS, trace=trace)
    if trace:
        global last_stage_ns
        if last_stage_ns is None:
            last_stage_ns = {}
        last_stage_ns[label] = res.exec_time_ns
    return res


def kernel(**inputs):
    global last_hw_ns, last_stage_ns
    last_stage_ns = None
    f32 = np.float32
    x = np.asarray(inputs["x"], f32)
    conv_w = np.asarray(inputs["conv_w"], f32)
    conv_b = np.asarray(inputs["conv_b"], f32)
    w1 = np.asarray(inputs["w1"], f32)
    b1 = np.asarray(inputs["b1"], f32)
    g1, be1 = np.asarray(inputs["g1"], f32), np.asarray(inputs["be1"], f32)
    m1, v1 = np.asarray(inputs["m1"], f32), np.asarray(inputs["v1"], f32)
    w2 = np.asarray(inputs["w2"], f32)
    b2 = np.asarray(inputs["b2"], f32)
    g2, be2 = np.asarray(inputs["g2"], f32), np.asarray(inputs["be2"], f32)
    m2, v2 = np.asarray(inputs["m2"], f32), np.asarray(inputs["v2"], f32)
    w3 = np.asarray(inputs["w3"], f32)
    b3 = np.asarray(inputs["b3"], f32)

    # ---- stage 1: ConvLSTM (batch-parallel)
    nc1 = _build_stage1()
    maps1 = _prep_stage1_inputs(x, conv_w, conv_b)
    res1 = _run(nc1, maps1, "stage1")
    import ml_dtypes
    bf = ml_dtypes.bfloat16
    hs_all = np.stack([res1.results[c]["hs"] for c in range(NC)])  # (8,S,H,32,L) bf16

    # ---- reshard: (8,S,H,32,L) -> flatT (S*H*L, 256), feature-major, bf16
    flatT = np.ascontiguousarray(
        hs_all.transpose(1, 2, 4, 0, 3)
    ).reshape(KTOT, B)

    # ---- stage 2: big GEMM, contract-dim sharded
    s1 = g1 / np.sqrt(v1 + EPS)
    c1 = b1 * s1 + (be1 - m1 * s1)
    w1sT = np.ascontiguousarray((w1 * s1[:, None]).T).astype(bf)    # (KTOT, 768)
    nc2 = _build_stage2()
    maps2 = []
    for c in range(NC):
        sl = slice(c * KSH, (c + 1) * KSH)
        maps2.append({
            "w1p": np.ascontiguousarray(w1sT[sl]).reshape(KCH, 128, 768),
            "ft": np.ascontiguousarray(flatT[sl]).reshape(KCH, 128, B),
        })
    res2 = _run(nc2, maps2, "stage2")
    y1 = np.sum([res2.results[c]["y1p"] for c in range(NC)], axis=0,
                dtype=np.float64).astype(f32)                       # (768, 256)

    # ---- stage 3: epilogue (batch-parallel)
    s2 = g2 / np.sqrt(v2 + EPS)
    c2 = b2 * s2 + (be2 - m2 * s2)
    c1t = np.ascontiguousarray(c1.reshape(6, 128).T, f32)           # (128, 6)
    w2p = np.ascontiguousarray(
        (w2 * s2[:, None]).T.reshape(6, 128, 12), f32
    )
    w3e = np.concatenate([w3.T, b3[None, :]], axis=0).astype(f32)   # (13, 10)
    nc3 = _build_stage3()
    maps3 = []
    for c in range(NC):
        ysl = np.ascontiguousarray(
            y1[:, c * BLOC : (c + 1) * BLOC]
        ).reshape(6, 128, BLOC)
        maps3.append({
            "y1s": ysl, "c1t": c1t, "w2p": w2p,
            "c2t": c2.reshape(12, 1).astype(f32), "w3e": w3e,
        })
    res3 = _run(nc3, maps3, "stage3")
    y3 = np.concatenate([res3.results[c]["y3p"] for c in range(NC)], axis=0)
    if last_stage_ns and all(v is not None for v in last_stage_ns.values()):
        last_hw_ns = sum(last_stage_ns.values())
    return np.ascontiguousarray(y3, f32)
